# revision 4
# baseline (speedup 1.0000x reference)
"""im2col (3x3, SAME zero padding) kernel for Trainium2.

Full op: x (16, 64, 128, 128) f32 -> out (16, 128, 128, 64, 3, 3) f32 with
    out[b, h, w, c, i, j] = pad(x)[b, c, h + i, w + j]   (pad = 1 px zeros)

Data-parallel over batch: 8 cores x 2 images, no cross-device traffic.
h-on-partitions layout; window-shared shift matmuls (2 per chunk).

Per w-chunk of 6, one matmul per i-shift covers the w-window [w0-1,
w0+7) (width <= 8, so C*ww <= 512 stays in one PSUM bank); the three
j-offset copies read different sub-windows of the same PSUM tile.  This
cuts TensorE work ~2.7x -- relevant because the PE HAM clock gate holds
the array at half rate for sparse activity, putting the old 6-matmul
chain near the store-DMA critical path.
"""

import sys

for _p in ("/opt/trn_rl_repo", "/root/.axon_site/_ro/trn_rl_repo"):
    if _p not in sys.path:
        sys.path.append(_p)

import numpy as np

import concourse.bacc as bacc
import concourse.mybir as mybir
from concourse import bass_utils
from concourse.tile import TileContext

F32 = mybir.dt.float32
BF16 = mybir.dt.bfloat16

B, C, H, W = 16, 64, 128, 128
KS = 3
N_CORES = 8
B_LOC = B // N_CORES

CW = 6          # w-chunk; window CW+2 = 8 -> C*(CW+2) = 512 = one bank
F = C * KS * KS


def _make_shifted_identity(nc, tile, base):
    nc.gpsimd.memset(tile, 0.0)
    nc.gpsimd.affine_select(
        out=tile,
        in_=tile,
        compare_op=mybir.AluOpType.not_equal,
        fill=1.0,
        base=base,
        pattern=[[-1, tile.shape[1]]],
        channel_multiplier=1,
    )


def _build_kernel(n_b: int = B_LOC, repeat: int = 1,
                  xin_bufs: int = 2, osb_bufs: int = 4, ps_bufs: int = 8):
    nc = bacc.Bacc("TRN2", target_bir_lowering=False, debug=False)

    x = nc.dram_tensor("x", (n_b, C, H, W), F32, kind="ExternalInput")
    out = nc.dram_tensor("out", (n_b, H, W, C, KS, KS), F32, kind="ExternalOutput")
    x_ap = x.ap()
    out_ap = out.ap()

    chunks = []
    w0 = 0
    while w0 < W:
        chunks.append((w0, min(CW, W - w0)))
        w0 += CW

    with TileContext(nc) as tc:
        with (
            tc.tile_pool(name="const", bufs=1) as const_pool,
            tc.tile_pool(name="xin", bufs=xin_bufs) as xin_pool,
            tc.tile_pool(name="xb", bufs=xin_bufs) as xb_pool,
            tc.tile_pool(name="ps", bufs=ps_bufs, space="PSUM") as psum_pool,
            tc.tile_pool(name="osb", bufs=osb_bufs) as osb_pool,
        ):
            p_dn = const_pool.tile([H, H], BF16)
            p_up = const_pool.tile([H, H], BF16)
            _make_shifted_identity(nc, p_dn, base=1)
            _make_shifted_identity(nc, p_up, base=-1)
            shift_mat = {0: p_dn, 2: p_up}

            imgs = [(r, b) for r in range(repeat) for b in range(n_b)]
            xin_tiles = {}

            def load_img(idx):
                r, b = imgs[idx]
                t = xin_pool.tile([H, C, W], F32)
                nc.scalar.dma_start(
                    out=t, in_=x_ap[b].rearrange("c h w -> h c w")
                )
                tb = xb_pool.tile([H, C, W], BF16)
                nc.vector.tensor_copy(tb, t)
                xin_tiles[idx] = (t, tb)

            load_img(0)
            for idx, (r, b) in enumerate(imgs):
                xin, xb = xin_tiles.pop(idx)
                for ci, (w0, cw) in enumerate(chunks):
                    if ci == len(chunks) // 2 and idx + 1 < len(imgs):
                        load_img(idx + 1)

                    osb = osb_pool.tile([H, cw, C, KS, KS], F32)
                    win_lo = max(w0 - 1, 0)
                    win_hi = min(w0 + cw + 1, W)
                    ww = win_hi - win_lo
                    for i in (0, 2):
                        ps = psum_pool.tile([H, C, ww], F32)
                        nc.tensor.matmul(
                            ps,
                            shift_mat[i][:, :],
                            xb[:, :, win_lo:win_hi],
                            start=True,
                            stop=True,
                        )
                        for j in range(KS):
                            a = w0 + j - 1
                            lo = max(a, 0)
                            hi = min(a + cw, W)
                            n = hi - lo
                            d0 = lo - a
                            dst = osb[:, d0:d0 + n, :, i, j]
                            src = ps[:, :, lo - win_lo:hi - win_lo].rearrange(
                                "p c w -> p w c"
                            )
                            if i == 0:
                                nc.vector.tensor_copy(dst, src)
                            else:
                                nc.scalar.copy(dst, src)
                    for j in range(KS):
                        a = w0 + j - 1
                        lo = max(a, 0)
                        hi = min(a + cw, W)
                        n = hi - lo
                        d0 = lo - a
                        dst = osb[:, d0:d0 + n, :, 1, j]
                        src = xin[:, :, lo:hi].rearrange("p c w -> p w c")
                        nc.gpsimd.tensor_copy(dst, src)
                    if w0 == 0:
                        nc.gpsimd.memset(osb[:, 0, :, :, 0], 0.0)
                    if w0 + cw == W:
                        nc.gpsimd.memset(osb[:, cw - 1, :, :, 2], 0.0)

                    st_eng = nc.sync if ci % 2 == 0 else nc.scalar
                    st_eng.dma_start(
                        out=out_ap[b].rearrange("h w c i j -> h w (c i j)")[
                            :, w0:w0 + cw, :
                        ],
                        in_=osb.rearrange("p w c i j -> p w (c i j)"),
                    )

    nc.compile()
    return nc


_NC_CACHE = {}


def _get_nc(n_b: int):
    if n_b not in _NC_CACHE:
        _NC_CACHE[n_b] = _build_kernel(n_b)
    return _NC_CACHE[n_b]


def run_spmd(x: np.ndarray, **kwargs) -> bass_utils.BassKernelResults:
    x = np.ascontiguousarray(np.asarray(x, dtype=np.float32))
    assert x.shape == (B, C, H, W), x.shape
    nc = _get_nc(B_LOC)
    in_maps = [
        {"x": x[i * B_LOC : (i + 1) * B_LOC]} for i in range(N_CORES)
    ]
    return bass_utils.run_bass_kernel_spmd(
        nc, in_maps, core_ids=list(range(N_CORES)), **kwargs
    )


def kernel(x: np.ndarray) -> np.ndarray:
    res = run_spmd(x)
    return np.concatenate([r["out"] for r in res.results], axis=0)
